# revision 7
# baseline (speedup 1.0000x reference)
"""Causal self-attention (B=2, T=2048, C=1024, H=16) on 8 TRN2 NeuronCores.

Sharding: 8 cores = 2 batches x 4 head-groups (4 heads each).
Each core computes qkv for its heads, causal attention, and a partial
output projection; the host sums the 4 partial projections per batch.

All matmuls run in float32r (TF32-like) at 1 cycle/row.
"""

import numpy as np

import concourse.bass as bass
import concourse.mybir as mybir
import concourse.tile as tile
from concourse import bacc, bass_utils
from concourse.masks import make_identity, make_upper_triangular

F32 = mybir.dt.float32
F32R = mybir.dt.float32r
AF = mybir.ActivationFunctionType

B = 2
T = 2048
C = 1024
H = 16
D = 64
N_CORES = 8
HG = 4            # heads per core
CG = HG * D       # 256 y-columns per core
P = 128
TQ = 512          # q block width
NKT = T // P      # 16 k tiles
NQB = T // TQ     # 4 q blocks
NCT = C // P      # 8 contraction tiles for qkv
NM = 3 * CG // P  # 6 output m-tiles for qkvT (q:2, k:2, v:2)

_cached = {}


def _build_nc():
    nc = bacc.Bacc("TRN2", target_bir_lowering=False, debug=False,
                   num_devices=N_CORES)
    xT = nc.dram_tensor("xT", [C, T], F32, kind="ExternalInput")
    wT = nc.dram_tensor("wT", [C, 3 * CG], F32, kind="ExternalInput")
    pT = nc.dram_tensor("pT", [CG, C], F32, kind="ExternalInput")
    out = nc.dram_tensor("out", [T, C], F32, kind="ExternalOutput")

    with tile.TileContext(nc) as tc:
        with (
            tc.tile_pool(name="const", bufs=1) as const,
            tc.tile_pool(name="persist", bufs=1) as persist,
        ):
            # --- constants ---
            tmpf = const.tile([P, P], F32, tag="tmpf")
            make_identity(nc, tmpf[:])
            ident = const.tile([P, P], F32R, tag="ident")
            nc.vector.tensor_copy(ident[:], tmpf[:])

            tmpf2 = const.tile([P, P], F32, tag="tmpf2")
            make_upper_triangular(nc, tmpf2[:], val=1.0, diag=True)
            tri = const.tile([P, P], F32R, tag="tri")
            nc.vector.tensor_copy(tri[:], tmpf2[:])

            onesf = const.tile([P, D], F32, tag="onesf")
            nc.vector.memset(onesf[:], 1.0)
            ones_lhsT = const.tile([1, D], F32R, tag="ones_lhsT")
            nc.vector.tensor_copy(ones_lhsT[:], onesf[:1, :D])

            # --- persistent tensors ---
            qkvT = [persist.tile([P, T], F32R, tag=f"qkvT{m}", name=f"qkvT{m}") for m in range(NM)]
            # v_aug per head: 16 k-tiles of [128, 65] (64 v cols + ones col)
            vaug = [persist.tile([P, NKT * (D + 1)], F32R, tag=f"vaug{h}", name=f"vaug{h}")
                    for h in range(HG)]
            yT = [persist.tile([P, T], F32R, tag=f"yT{j}", name=f"yT{j}") for j in range(2)]
            wp = [persist.tile([P, C], F32R, tag=f"wp{j}", name=f"wp{j}") for j in range(2)]

            for j in range(2):
                nc.sync.dma_start(wp[j][:], pT[j * P:(j + 1) * P, :].bitcast(F32R))

            # ================= stage A: qkvT = W_g @ x^T =================
            with (
                tc.tile_pool(name="xw", bufs=1) as xw,
                tc.tile_pool(name="psA", bufs=4, space="PSUM") as psA,
                tc.tile_pool(name="psT", bufs=2, space="PSUM") as psT,
            ):
                xt = [xw.tile([P, T], F32R, tag=f"x{c}", name=f"x{c}") for c in range(NCT)]
                wt = [xw.tile([P, 3 * CG], F32R, tag=f"w{c}", name=f"w{c}") for c in range(NCT)]
                for c in range(NCT):
                    nc.sync.dma_start(xt[c][:], xT[c * P:(c + 1) * P, :].bitcast(F32R))
                    nc.sync.dma_start(wt[c][:], wT[c * P:(c + 1) * P, :].bitcast(F32R))

                for n in range(T // TQ):
                    for m in range(NM):
                        ps = psA.tile([P, TQ], F32, tag="ps")
                        for c in range(NCT):
                            nc.tensor.matmul(
                                ps[:],
                                wt[c][:, m * P:(m + 1) * P],
                                xt[c][:, n * TQ:(n + 1) * TQ],
                                start=(c == 0), stop=(c == NCT - 1),
                            )
                        nc.vector.tensor_copy(qkvT[m][:, n * TQ:(n + 1) * TQ], ps[:])

                # ============ stage B: v natural layout + ones column ============
                for j in range(2):  # v m-tiles: heads (2j, 2j+1)
                    for kt in range(NKT):
                        pt = psT.tile([P, P], F32R, tag="pt")
                        nc.tensor.transpose(
                            pt[:], qkvT[4 + j][:, kt * P:(kt + 1) * P], ident[:])
                        for hh in range(2):
                            h = 2 * j + hh
                            nc.vector.tensor_copy(
                                vaug[h][:, kt * (D + 1):kt * (D + 1) + D],
                                pt[:, hh * D:(hh + 1) * D])
                for h in range(HG):
                    # ones columns at offset D within each 65-wide group
                    nc.vector.tensor_copy(
                        vaug[h][:, D::(D + 1)], onesf[:, :NKT])

            # ================= stages C+D: attention + proj =================
            with (
                tc.tile_pool(name="esb", bufs=3) as esb,
                tc.tile_pool(name="small", bufs=2) as small,
                tc.tile_pool(name="osb", bufs=3) as osb,
                tc.tile_pool(name="psS", bufs=2, space="PSUM") as psS,
                tc.tile_pool(name="psAv", bufs=1, space="PSUM") as psAv,
                tc.tile_pool(name="psX", bufs=2, space="PSUM") as psX,
            ):
                for qb in range(NQB):
                    q0 = qb * TQ
                    nkt = (qb + 1) * (TQ // P)
                    for j in range(2):  # head pair (2j, 2j+1)
                        qm, km = qkvT[j], qkvT[2 + j]
                        avp = [psAv.tile([D + 1, TQ], F32, tag=f"av{hh}", name=f"av{hh}_{qb}_{j}")
                               for hh in range(2)]
                        for kt in range(nkt):
                            z = max(0, kt * P - q0)
                            sc = [psS.tile([P, TQ], F32, tag=f"sc{hh}", name=f"sc{hh}_{qb}_{j}_{kt}")
                                  for hh in range(2)]
                            ee = [esb.tile([P, TQ], F32R, tag=f"e{hh}", name=f"e{hh}_{qb}_{j}_{kt}")
                                  for hh in range(2)]
                            for hh in range(2):
                                nc.tensor.matmul(
                                    sc[hh][:, z:],
                                    km[hh * D:(hh + 1) * D, kt * P:(kt + 1) * P],
                                    qm[hh * D:(hh + 1) * D, q0 + z:q0 + TQ],
                                    start=True, stop=True,
                                    tile_position=(hh * D, 0),
                                )
                                nc.scalar.activation(
                                    ee[hh][:, z:], sc[hh][:, z:], AF.Exp,
                                    scale=0.125)
                                if kt * P >= q0:  # diagonal band tile
                                    nc.vector.tensor_mul(
                                        ee[hh][:, z:z + P],
                                        ee[hh][:, z:z + P], tri[:])
                                nc.tensor.matmul(
                                    avp[hh][:, z:],
                                    vaug[2 * j + hh][:, kt * (D + 1):(kt + 1) * (D + 1)],
                                    ee[hh][:, z:],
                                    start=(kt == 0), stop=(kt == nkt - 1),
                                )
                        # softmax division: y = av / sums
                        for hh in range(2):
                            rp = small.tile([1, TQ], F32R, tag="rp")
                            with nc.allow_low_precision(reason="f32r recip"):
                                nc.vector.reciprocal(rp[:], avp[hh][D:D + 1, :])
                            bc = psX.tile([D, TQ], F32, tag="pp", name=f"bc_{qb}_{j}_{hh}")
                            nc.tensor.matmul(bc[:], ones_lhsT[:], rp[:],
                                             start=True, stop=True)
                            bcs = small.tile([D, TQ], F32R, tag="bcs")
                            nc.vector.tensor_copy(bcs[:], bc[:])
                            nc.vector.tensor_mul(
                                yT[j][hh * D:(hh + 1) * D, q0:q0 + TQ],
                                avp[hh][:D, :], bcs[:])

                    # ---- stage D: proj for this q block ----
                    for tb in range(qb * (TQ // P), (qb + 1) * (TQ // P)):
                        for oh in range(2):
                            pp = psX.tile([P, TQ], F32, tag="pp")
                            for cc in range(2):
                                nc.tensor.matmul(
                                    pp[:],
                                    yT[cc][:, tb * P:(tb + 1) * P],
                                    wp[cc][:, oh * TQ:(oh + 1) * TQ],
                                    start=(cc == 0), stop=(cc == 1),
                                )
                            ob = osb.tile([P, TQ], F32, tag="ob")
                            nc.vector.tensor_copy(ob[:], pp[:])
                            nc.sync.dma_start(
                                out[tb * P:(tb + 1) * P, oh * TQ:(oh + 1) * TQ],
                                ob[:])

    nc.compile()
    return nc


def _prep_inputs(x, w_qkv, w_proj):
    """Build per-core input maps. Core c = b * 4 + hg."""
    in_maps = []
    xTb = [np.ascontiguousarray(x[b].T) for b in range(B)]
    for b in range(B):
        for hg in range(HG):
            rows = np.concatenate([
                np.arange(hg * CG, (hg + 1) * CG),            # q rows
                np.arange(C + hg * CG, C + (hg + 1) * CG),    # k rows
                np.arange(2 * C + hg * CG, 2 * C + (hg + 1) * CG),  # v rows
            ])
            wTg = np.ascontiguousarray(w_qkv[rows].T)         # [C, 3*CG]
            pTg = np.ascontiguousarray(w_proj[:, hg * CG:(hg + 1) * CG].T)
            in_maps.append({"xT": xTb[b], "wT": wTg, "pT": pTg})
    return in_maps


def kernel(x, w_qkv, w_proj):
    x = np.asarray(x, dtype=np.float32)
    w_qkv = np.asarray(w_qkv, dtype=np.float32)
    w_proj = np.asarray(w_proj, dtype=np.float32)

    if "nc" not in _cached:
        _cached["nc"] = _build_nc()
    nc = _cached["nc"]

    in_maps = _prep_inputs(x, w_qkv, w_proj)
    res = bass_utils.run_bass_kernel_spmd(nc, in_maps, core_ids=list(range(N_CORES)))

    out = np.zeros((B, T, C), dtype=np.float32)
    for b in range(B):
        for hg in range(HG):
            out[b] += res.results[b * HG + hg]["out"]
    return out


# revision 9
# speedup vs baseline: 1.1184x; 1.1184x over previous
"""Causal self-attention (B=2, T=2048, C=1024, H=16) on 8 TRN2 NeuronCores.

Sharding: 8 cores = 2 batches x 4 head-groups (4 heads each).
Each core computes qkv for its heads, causal attention, and a partial
output projection; the host sums the 4 partial projections per batch.

All matmuls run in float32r (TF32-like) at 1 cycle/row.

Layouts (per core):
  xT   [C, T]        x^T, streamed in [128, 512] slices
  wT   [C, 768]      qkv weight slice, pre-transposed (lhsT)
  qkvT [6][128, T]   m0,m1 = q^T (heads 01, 23); m2,m3 = k^T; m4,m5 = v^T
  vaug [2][128, 2080] per head pair: 16 k-tile groups of 130 cols =
                     [v_h0 (64) | ones | v_h1 (64) | ones]
  attention in scoresT layout: partition = k, free = q. exp on ScalarE
  handles both heads of a pair in one instruction (2-bank PSUM tile).
  av^T accumulated via matmul with v_aug (ones column -> softmax sums).
"""

import numpy as np

import concourse.bass as bass
import concourse.mybir as mybir
import concourse.tile as tile
from concourse import bacc, bass_utils
from concourse.masks import make_identity, make_upper_triangular

F32 = mybir.dt.float32
F32R = mybir.dt.float32r
AF = mybir.ActivationFunctionType

B = 2
T = 2048
C = 1024
D = 64
N_CORES = 8
HG = 4            # heads per core
CG = HG * D       # 256 y-columns per core
P = 128
TQ = 512          # q block width
NKT = T // P      # 16 k tiles
NQB = T // TQ     # 4 q blocks
NCT = C // P      # 8 contraction tiles for qkv
NM = 3 * CG // P  # 6 output m-tiles for qkvT

_cached = {}


def _build_nc():
    nc = bacc.Bacc("TRN2", target_bir_lowering=False, debug=False,
                   num_devices=N_CORES)
    xT = nc.dram_tensor("xT", [C, T], F32, kind="ExternalInput")
    wT = nc.dram_tensor("wT", [C, 3 * CG], F32, kind="ExternalInput")
    pT = nc.dram_tensor("pT", [CG, C], F32, kind="ExternalInput")
    out = nc.dram_tensor("out", [T, C], F32, kind="ExternalOutput")

    with tile.TileContext(nc) as tc:
        with (
            tc.tile_pool(name="const", bufs=1) as const,
            tc.tile_pool(name="persist", bufs=1) as persist,
            tc.tile_pool(name="xw", bufs=2) as xw,
            tc.tile_pool(name="wpool", bufs=1) as wpool,
            tc.tile_pool(name="esb", bufs=3) as esb,
            tc.tile_pool(name="small", bufs=2) as small,
            tc.tile_pool(name="psSC", bufs=2, space="PSUM") as psSC,
            tc.tile_pool(name="psMM", bufs=2, space="PSUM") as psMM,
            tc.tile_pool(name="psAv", bufs=1, space="PSUM") as psAv,
        ):
            # ---------------- constants ----------------
            tmpf = const.tile([P, 2 * P], F32, tag="tmpf")
            make_identity(nc, tmpf[:, :P])
            make_upper_triangular(nc, tmpf[:, P:], val=1.0, diag=True)
            ident = const.tile([P, P], F32R, tag="ident")
            nc.vector.tensor_copy(ident[:], tmpf[:, :P])
            # tri2: the [pk <= fq] mask, duplicated for the two heads
            tri2 = const.tile([P, 2 * P], F32R, tag="tri2")
            nc.vector.tensor_copy(tri2[:, :P], tmpf[:, P:])
            nc.vector.tensor_copy(tri2[:, P:], tmpf[:, P:])

            onesf = const.tile([P, D], F32, tag="onesf")
            nc.vector.memset(onesf[:], 1.0)
            ones_lhsT = const.tile([1, D], F32R, tag="ones_lhsT")
            nc.vector.tensor_copy(ones_lhsT[:], onesf[:1, :])

            # ---------------- persistent tensors ----------------
            qkvT = [persist.tile([P, T], F32R, tag=f"qkvT{m}", name=f"qkvT{m}")
                    for m in range(NM)]
            vaug = [persist.tile([P, NKT * 2 * (D + 1)], F32R,
                                 tag=f"vaug{j}", name=f"vaug{j}")
                    for j in range(2)]
            yT = [persist.tile([P, T], F32R, tag=f"yT{j}", name=f"yT{j}")
                  for j in range(2)]
            wp = [persist.tile([P, C], F32R, tag=f"wp{j}", name=f"wp{j}")
                  for j in range(2)]

            # ---------------- input DMAs ----------------
            # x slices for n=0 first so the first matmul chains start early
            xs = {}
            for n in range(NQB):
                for c in range(NCT):
                    t0 = n * TQ
                    xs[(c, n)] = xw.tile([P, TQ], F32R, tag=f"x{c}",
                                         name=f"x{c}_{n}")
                    nc.sync.dma_start(
                        xs[(c, n)][:],
                        xT[c * P:(c + 1) * P, t0:t0 + TQ].bitcast(F32R))
                if n == 0:
                    wt = []
                    for c in range(NCT):
                        w_ = wpool.tile([P, 3 * CG], F32R, tag=f"w{c}",
                                        name=f"w{c}")
                        wt.append(w_)
                        nc.sync.dma_start(
                            w_[:], wT[c * P:(c + 1) * P, :].bitcast(F32R))
                    for j in range(2):
                        nc.sync.dma_start(
                            wp[j][:],
                            pT[j * P:(j + 1) * P, :].bitcast(F32R))

            # ---------------- stage A: qkvT = W_g @ x^T ----------------
            for n in range(NQB):
                for m in range(NM):
                    ps = psMM.tile([P, TQ], F32, tag="mm", name=f"psA_{n}_{m}")
                    for c in range(NCT):
                        nc.tensor.matmul(
                            ps[:],
                            wt[c][:, m * P:(m + 1) * P],
                            xs[(c, n)][:],
                            start=(c == 0), stop=(c == NCT - 1),
                        )
                    nc.vector.tensor_copy(qkvT[m][:, n * TQ:(n + 1) * TQ], ps[:])

            # ------- stage B: v to natural layout (paired) + ones -------
            for j in range(2):
                vr = vaug[j].rearrange("p (k g x) -> p k g x", k=NKT, g=2)
                for kt in range(NKT):
                    pt = psMM.tile([P, P], F32R, tag="mm", name=f"pt_{j}_{kt}")
                    nc.tensor.transpose(
                        pt[:], qkvT[4 + j][:, kt * P:(kt + 1) * P], ident[:])
                    # one copy: [128,128] -> cols {0..63, 65..128} of group kt
                    nc.vector.tensor_copy(
                        vr[:, kt, :, :D],
                        pt[:].rearrange("p (g x) -> p g x", g=2))
                # ones columns at 64 + 65*m for m in 0..31
                nc.vector.tensor_copy(vaug[j][:, D::D + 1], onesf[:, :2 * NKT])

            # ---------------- stages C+D: attention + proj ----------------
            for qb in range(NQB):
                q0 = qb * TQ
                nkt = (qb + 1) * (TQ // P)
                for j in range(2):  # head pair (2j, 2j+1)
                    qm, km = qkvT[j], qkvT[2 + j]
                    vr = vaug[j].rearrange("p (k g x) -> p k g x", k=NKT, g=2)
                    avp = [psAv.tile([D + 1, TQ], F32, tag=f"av{hh}",
                                     name=f"av{hh}_{qb}_{j}")
                           for hh in range(2)]
                    for kt in range(nkt):
                        z = max(0, kt * P - q0)
                        sc = psSC.tile([P, 2 * TQ], F32, tag="sc",
                                       name=f"sc_{qb}_{j}_{kt}")
                        scr = sc.rearrange("p (g x) -> p g x", g=2)
                        ee = esb.tile([P, 2 * TQ], F32R, tag="ee",
                                      name=f"ee_{qb}_{j}_{kt}")
                        eer = ee.rearrange("p (g x) -> p g x", g=2)
                        for hh in range(2):
                            nc.tensor.matmul(
                                scr[:, hh, z:],
                                km[hh * D:(hh + 1) * D, kt * P:(kt + 1) * P],
                                qm[hh * D:(hh + 1) * D, q0 + z:q0 + TQ],
                                start=True, stop=True,
                                tile_position=(hh * D, 0),
                            )
                        # exp for both heads in one instruction
                        nc.scalar.activation(
                            eer[:, :, z:], scr[:, :, z:], AF.Exp, scale=0.125)
                        if kt * P >= q0:  # diagonal band: triangular mask
                            nc.vector.tensor_mul(
                                eer[:, :, z:z + P], eer[:, :, z:z + P],
                                tri2[:].rearrange("p (g x) -> p g x", g=2))
                        for hh in range(2):
                            nc.tensor.matmul(
                                avp[hh][:, z:],
                                vr[:, kt, hh, :],
                                eer[:, hh, z:],
                                start=(kt == 0), stop=(kt == nkt - 1),
                            )
                    # softmax division: yT = avT * (1/sums) broadcast
                    for hh in range(2):
                        rp = small.tile([1, TQ], F32R, tag="rp",
                                        name=f"rp_{qb}_{j}_{hh}")
                        with nc.allow_low_precision(reason="f32r recip"):
                            nc.vector.reciprocal(rp[:], avp[hh][D:D + 1, :])
                        bc = psMM.tile([D, TQ], F32, tag="mm",
                                       name=f"bc_{qb}_{j}_{hh}")
                        nc.tensor.matmul(bc[:], ones_lhsT[:], rp[:],
                                         start=True, stop=True)
                        bcs = small.tile([D, TQ], F32R, tag="bcs",
                                         name=f"bcs_{qb}_{j}_{hh}")
                        nc.vector.tensor_copy(bcs[:], bc[:])
                        nc.vector.tensor_mul(
                            yT[j][hh * D:(hh + 1) * D, q0:q0 + TQ],
                            avp[hh][:D, :], bcs[:])

                # ---- stage D: proj for this q block (DMA from PSUM) ----
                for tb in range(qb * (TQ // P), (qb + 1) * (TQ // P)):
                    for oh in range(2):
                        pp = psMM.tile([P, TQ], F32, tag="mm",
                                       name=f"pp_{tb}_{oh}")
                        for cc in range(2):
                            nc.tensor.matmul(
                                pp[:],
                                yT[cc][:, tb * P:(tb + 1) * P],
                                wp[cc][:, oh * TQ:(oh + 1) * TQ],
                                start=(cc == 0), stop=(cc == 1),
                            )
                        ob = esb.tile([P, TQ], F32, tag="ob",
                                      name=f"ob_{tb}_{oh}")
                        nc.vector.tensor_copy(ob[:], pp[:])
                        nc.sync.dma_start(
                            out[tb * P:(tb + 1) * P, oh * TQ:(oh + 1) * TQ],
                            ob[:])

    nc.compile()
    return nc


def _prep_inputs(x, w_qkv, w_proj):
    """Build per-core input maps. Core c = b * 4 + hg."""
    in_maps = []
    xTb = [np.ascontiguousarray(x[b].T) for b in range(B)]
    for b in range(B):
        for hg in range(HG):
            sl = slice(hg * CG, (hg + 1) * CG)
            wTg = np.ascontiguousarray(
                np.concatenate([w_qkv[sl], w_qkv[C:][sl], w_qkv[2 * C:][sl]],
                               axis=0).T)
            pTg = np.ascontiguousarray(w_proj[:, sl].T)
            in_maps.append({"xT": xTb[b], "wT": wTg, "pT": pTg})
    return in_maps


def kernel(x, w_qkv, w_proj):
    x = np.asarray(x, dtype=np.float32)
    w_qkv = np.asarray(w_qkv, dtype=np.float32)
    w_proj = np.asarray(w_proj, dtype=np.float32)

    if "nc" not in _cached:
        _cached["nc"] = _build_nc()
    nc = _cached["nc"]

    in_maps = _prep_inputs(x, w_qkv, w_proj)
    res = bass_utils.run_bass_kernel_spmd(nc, in_maps, core_ids=list(range(N_CORES)))

    out = np.zeros((B, T, C), dtype=np.float32)
    for b in range(B):
        for hg in range(HG):
            out[b] += res.results[b * HG + hg]["out"]
    return out
